# revision 3
# baseline (speedup 1.0000x reference)
"""Trainium2 Bass kernel for nn_CrossAttentionBridge.

The reference module is a cross-attention bridge with q_len = kv_len = 1.
Softmax over a single key is identically 1, so `attn = v2` and the whole
q/k path is dead code.  The module collapses to a single affine map:

    out = vit_feat @ (Wo @ Wiv @ Wv).T + (Wo @ (Wiv @ bv + biv) + bo)

where Wiv/biv are the v-slice of in_proj.  We fold the weights on the host
(float64) and run one [B,512] @ [512,1024] matmul on 8 NeuronCores, batch
(data) parallel: 2048 rows per core.

Device layout per core:
  xt   [512, 2048] f32  - vit_feat shard, pre-transposed so the contraction
                          dim (512) lands on SBUF partitions (4 chunks of 128)
  wc   [512, 1024] f32  - folded weight, (Wo@Wiv@Wv).T
  bias [128, 1024] f32  - folded bias broadcast across partitions
  y    [2048, 1024] f32 - output shard

Per 128-row output tile: 8 matmuls (4 K-chunks x 2 PSUM halves of N=512)
accumulate into a [128,1024] PSUM tile; bias is fused into the PSUM->SBUF
eviction on the vector engine; the tile is stored with one 512 KB DMA.
"""

import numpy as np
from contextlib import ExitStack

B = 16384
VIT_D = 512
E = 1024
N_CORES = 8
ROWS = B // N_CORES  # 2048 rows per core
P = 128
KC = VIT_D // P      # 4 contraction chunks
MT = ROWS // P       # 16 output row tiles per core
NF = 512             # moving-operand free dim per matmul
NH = E // NF         # 2 PSUM halves

_CACHE = {}


def _build_bass():
    import concourse.bacc as bacc
    import concourse.tile as tile
    import concourse.mybir as mybir

    nc = bacc.Bacc()
    xt = nc.declare_dram_parameter("xt", [VIT_D, ROWS], mybir.dt.float32, isOutput=False)
    wc = nc.declare_dram_parameter("wc", [VIT_D, E], mybir.dt.float32, isOutput=False)
    bias = nc.declare_dram_parameter("bias", [P, E], mybir.dt.float32, isOutput=False)
    y = nc.declare_dram_parameter("y", [ROWS, E], mybir.dt.float32, isOutput=True)

    with ExitStack() as ctx:
        tc = ctx.enter_context(tile.TileContext(nc))
        xt_pool = ctx.enter_context(tc.tile_pool(name="xt_pool", bufs=KC))
        wc_pool = ctx.enter_context(tc.tile_pool(name="wc_pool", bufs=KC))
        const_pool = ctx.enter_context(tc.tile_pool(name="const_pool", bufs=1))
        psum_pool = ctx.enter_context(tc.tile_pool(name="psum_pool", bufs=3, space="PSUM"))
        out_pool = ctx.enter_context(tc.tile_pool(name="out_pool", bufs=4))

        bias_t = const_pool.tile([P, E], mybir.dt.float32)
        nc.sync.dma_start(bias_t[:], bias[:, :])

        xt_tiles = []
        wc_tiles = []
        for k in range(KC):
            wct = wc_pool.tile([P, E], mybir.dt.float32)
            nc.sync.dma_start(wct[:], wc[k * P:(k + 1) * P, :])
            wc_tiles.append(wct)
            xtt = xt_pool.tile([P, ROWS], mybir.dt.float32)
            nc.sync.dma_start(xtt[:], xt[k * P:(k + 1) * P, :])
            xt_tiles.append(xtt)

        for m in range(MT):
            ps = psum_pool.tile([P, E], mybir.dt.float32)
            for k in range(KC):
                for nh in range(NH):
                    nc.tensor.matmul(
                        ps[:, nh * NF:(nh + 1) * NF],
                        xt_tiles[k][:, m * P:(m + 1) * P],
                        wc_tiles[k][:, nh * NF:(nh + 1) * NF],
                        start=(k == 0),
                        stop=(k == KC - 1),
                    )
            ot = out_pool.tile([P, E], mybir.dt.float32)
            nc.vector.tensor_add(ot[:], ps[:], bias_t[:])
            nc.sync.dma_start(y[m * P:(m + 1) * P, :], ot[:])

    nc.compile()
    return nc


def _get_nc():
    if "nc" not in _CACHE:
        _CACHE["nc"] = _build_bass()
    return _CACHE["nc"]


def _prepare_device_inputs(inputs):
    vit = np.asarray(inputs["vit_feat"], dtype=np.float32)
    ipw = np.asarray(inputs["in_proj_w"])
    ipb = np.asarray(inputs["in_proj_b"])
    Wv = np.asarray(inputs["Wv"], dtype=np.float64)
    bv = np.asarray(inputs["bv"], dtype=np.float64)
    Wiv = ipw[2 * E:3 * E].astype(np.float64)
    biv = ipb[2 * E:3 * E].astype(np.float64)
    Wo = np.asarray(inputs["Wo"], dtype=np.float64)
    bo = np.asarray(inputs["bo"], dtype=np.float64)

    Wc = Wo @ Wiv @ Wv                 # [E, VIT_D]
    bc = Wo @ (Wiv @ bv + biv) + bo    # [E]

    wc_dev = np.ascontiguousarray(Wc.T, dtype=np.float32)          # [512, 1024]
    bias_dev = np.ascontiguousarray(
        np.broadcast_to(bc.astype(np.float32), (P, E)))            # [128, 1024]
    xt_full = np.ascontiguousarray(vit.T)                          # [512, 16384]

    in_maps = [
        {
            "xt": np.ascontiguousarray(xt_full[:, c * ROWS:(c + 1) * ROWS]),
            "wc": wc_dev,
            "bias": bias_dev,
        }
        for c in range(N_CORES)
    ]
    return in_maps


def run_device(in_maps, trace=False):
    from concourse.bass_utils import run_bass_kernel_spmd

    nc = _get_nc()
    return run_bass_kernel_spmd(nc, in_maps, list(range(N_CORES)), trace=trace)


def kernel(**inputs):
    in_maps = _prepare_device_inputs(inputs)
    res = run_device(in_maps, trace=False)
    return np.concatenate([res.results[c]["y"] for c in range(N_CORES)], axis=0)


# revision 7
# speedup vs baseline: 2.1694x; 2.1694x over previous
"""Trainium2 Bass kernel for nn_CrossAttentionBridge.

The reference module is a cross-attention bridge with q_len = kv_len = 1.
Softmax over a single key is identically 1, so `attn = v2` and the whole
q/k path is dead code.  The module collapses to a single affine map:

    out = vit_feat @ (Wo @ Wiv @ Wv).T + (Wo @ (Wiv @ bv + biv) + bo)

where Wiv/biv are the v-slice of in_proj.  We fold the weights on the host
(float64) and run one [B,512] @ [512,1024] matmul on 8 NeuronCores, batch
(data) parallel: 2048 rows per core.

Device layout per core:
  xt   [512, 2048] f32  - vit_feat shard, pre-transposed so the contraction
                          dim (512) lands on SBUF partitions (4 chunks of 128)
  wc   [512, 1024] f32  - folded weight, (Wo@Wiv@Wv).T
  bias [128, 1024] f32  - folded bias broadcast across partitions
  y    [2048, 1024] f32 - output shard

Per 128-row output tile: 8 matmuls (4 K-chunks x 2 PSUM halves of N=512)
accumulate into a [128,1024] PSUM tile; bias is fused into the PSUM->SBUF
eviction on the vector engine; the tile is stored with one 512 KB DMA.
"""

import numpy as np
from contextlib import ExitStack

B = 16384
VIT_D = 512
E = 1024
N_CORES = 8
ROWS = B // N_CORES  # 2048 rows per core
P = 128
KC = VIT_D // P      # 4 contraction chunks
MT = ROWS // P       # 16 output row tiles per core
NF = 512             # moving-operand free dim per matmul
NH = E // NF         # 2 PSUM halves

_CACHE = {}


def _build_bass():
    import concourse.bacc as bacc
    import concourse.tile as tile
    import concourse.mybir as mybir

    nc = bacc.Bacc()
    # float32r: same 4-byte layout as f32, but the PE streams it at
    # 1 cycle/row (vs 4 for plain f32) when the moving dim is >= 256.
    xt = nc.declare_dram_parameter("xt", [VIT_D, ROWS], mybir.dt.float32r, isOutput=False)
    wc = nc.declare_dram_parameter("wc", [VIT_D, E], mybir.dt.float32r, isOutput=False)
    bias = nc.declare_dram_parameter("bias", [P, E], mybir.dt.float32, isOutput=False)
    y = nc.declare_dram_parameter("y", [ROWS, E], mybir.dt.float32, isOutput=True)

    with ExitStack() as ctx:
        tc = ctx.enter_context(tile.TileContext(nc))
        xt_pool = ctx.enter_context(tc.tile_pool(name="xt_pool", bufs=KC))
        wc_pool = ctx.enter_context(tc.tile_pool(name="wc_pool", bufs=KC))
        const_pool = ctx.enter_context(tc.tile_pool(name="const_pool", bufs=1))
        psum_pool = ctx.enter_context(tc.tile_pool(name="psum_pool", bufs=3, space="PSUM"))
        out_pool = ctx.enter_context(tc.tile_pool(name="out_pool", bufs=4))

        bias_t = const_pool.tile([P, E], mybir.dt.float32)
        nc.sync.dma_start(bias_t[:], bias[:, :])

        xt_tiles = []
        wc_tiles = []
        for k in range(KC):
            wct = wc_pool.tile([P, E], mybir.dt.float32r)
            nc.sync.dma_start(wct[:], wc[k * P:(k + 1) * P, :])
            wc_tiles.append(wct)
            xtt = xt_pool.tile([P, ROWS], mybir.dt.float32r)
            nc.sync.dma_start(xtt[:], xt[k * P:(k + 1) * P, :])
            xt_tiles.append(xtt)

        for m in range(MT):
            ps = psum_pool.tile([P, E], mybir.dt.float32)
            for k in range(KC):
                for nh in range(NH):
                    nc.tensor.matmul(
                        ps[:, nh * NF:(nh + 1) * NF],
                        xt_tiles[k][:, m * P:(m + 1) * P],
                        wc_tiles[k][:, nh * NF:(nh + 1) * NF],
                        start=(k == 0),
                        stop=(k == KC - 1),
                    )
            ot = out_pool.tile([P, E], mybir.dt.float32)
            nc.vector.tensor_add(ot[:], ps[:], bias_t[:])
            nc.sync.dma_start(y[m * P:(m + 1) * P, :], ot[:])

    nc.compile()
    return nc


def _get_nc():
    if "nc" not in _CACHE:
        _CACHE["nc"] = _build_bass()
    return _CACHE["nc"]


def _prepare_device_inputs(inputs):
    vit = np.asarray(inputs["vit_feat"], dtype=np.float32)
    ipw = np.asarray(inputs["in_proj_w"])
    ipb = np.asarray(inputs["in_proj_b"])
    Wv = np.asarray(inputs["Wv"], dtype=np.float64)
    bv = np.asarray(inputs["bv"], dtype=np.float64)
    Wiv = ipw[2 * E:3 * E].astype(np.float64)
    biv = ipb[2 * E:3 * E].astype(np.float64)
    Wo = np.asarray(inputs["Wo"], dtype=np.float64)
    bo = np.asarray(inputs["bo"], dtype=np.float64)

    Wc = Wo @ Wiv @ Wv                 # [E, VIT_D]
    bc = Wo @ (Wiv @ bv + biv) + bo    # [E]

    wc_dev = np.ascontiguousarray(Wc.T, dtype=np.float32)          # [512, 1024]
    bias_dev = np.ascontiguousarray(
        np.broadcast_to(bc.astype(np.float32), (P, E)))            # [128, 1024]
    xt_full = np.ascontiguousarray(vit.T)                          # [512, 16384]

    in_maps = [
        {
            "xt": np.ascontiguousarray(xt_full[:, c * ROWS:(c + 1) * ROWS]),
            "wc": wc_dev,
            "bias": bias_dev,
        }
        for c in range(N_CORES)
    ]
    return in_maps


def run_device(in_maps, trace=False):
    from concourse.bass_utils import run_bass_kernel_spmd

    nc = _get_nc()
    return run_bass_kernel_spmd(nc, in_maps, list(range(N_CORES)), trace=trace)


def kernel(**inputs):
    in_maps = _prepare_device_inputs(inputs)
    res = run_device(in_maps, trace=False)
    return np.concatenate([res.results[c]["y"] for c in range(N_CORES)], axis=0)
